# revision 6
# baseline (speedup 1.0000x reference)
"""Trainium2 Bass kernel for nn_CumulativeProbingDense.

Computation (see reference):
    h      = sum_l softmax(mixing_weights)[l] * x[:, l] * gamma   # [B, S, F]
    h1     = relu(h @ W1.T + b1)                                  # [B, S, H]
    h2     = relu(h1 @ W2.T + b2)                                 # [B, S, H]
    pooled = (h2 * mask).sum(S) / lengths                         # [B, H]
    logits = pooled @ Wl.T + bl                                   # [B, NL]

Sharding: pure data parallel over batch, 2 samples per core on 8 cores.
The dominant cost is streaming x (654 MB fp32) from HBM once.

Device strategy per core:
  - layer mix on the TensorE: PSUM-accumulated matmuls with a scaled
    identity as the stationary operand and x tiles (natural [token, feat]
    layout) as the moving operand -> h tile [128 t, 768 f]
  - PE transpose h tiles into hT [feat partitions, token free]
  - MLP matmuls with W1T/W2T chunks stationary, relu+bias on ScalarE
  - masked mean-pool with one fused DVE tensor_tensor_reduce against a
    host-prebuilt mask/length tile, then a tiny matmul for the logits
"""

import numpy as np

import concourse.bass as bass
import concourse.tile as tile
from concourse import mybir
from concourse.bass_utils import run_bass_kernel_spmd
from contextlib import ExitStack

F32 = mybir.dt.float32
F32R = mybir.dt.float32r

N_CORES = 8
B, L, S, F = 16, 13, 1024, 768
H, NL = 256, 7
B_LOC = B // N_CORES          # samples per core
P = 128                       # SBUF partitions
TT = S // P                   # token tiles per sample
FC = F // P                   # feature chunks of 128
HC = H // P                   # hidden chunks of 128

# matmul input dtype: float32r streams at 1 cycle/row (moving dim >= 256)
# vs plain float32's 4 cycles/row. Verified against the fp32 reference on
# hardware: rel err ~2e-7 (the PE's fp32r path is not meaningfully lossy
# for this problem).
MM_DT = F32R


def _split_excess_waits(nc, max_waits=1):
    """walrus (CoreV3) rejects instructions carrying more than a couple of
    sync waits (e.g. the TileContext exit drain). Hoist excess waits onto
    standalone NoOps inserted before the offending instruction."""
    n_fixed = 0
    for f in nc.m.functions:
        for bb in f.blocks:
            out, changed = [], False
            for inst in bb.instructions:
                si = getattr(inst, "sync_info", None)
                if si is not None and len(si.on_wait) > max_waits:
                    waits = list(si.on_wait)
                    for j, w in enumerate(waits[max_waits:]):
                        out.append(mybir.InstNoOp(
                            name=f"{inst.name}-wsplit{j}",
                            engine=inst.engine, ins=[], outs=[],
                            sync_info=mybir.SyncInfo(on_wait=[w], on_update=[]),
                        ))
                    inst.sync_info = mybir.SyncInfo(
                        on_wait=waits[:max_waits], on_update=list(si.on_update))
                    changed = True
                    n_fixed += 1
                out.append(inst)
            if changed:
                bb.instructions = out
    return n_fixed


def _r(ap):
    return ap


def build_program(n_layers: int, split_waits: bool = True) -> bass.Bass:
    nc = bass.Bass("TRN2", target_bir_lowering=False, debug=False, num_devices=1)

    x_d = nc.dram_tensor("x", [B_LOC, L, S, F], F32R, kind="ExternalInput").ap()
    seye_d = nc.dram_tensor("seye", [P, L * P], F32R, kind="ExternalInput").ap()
    ident_d = nc.dram_tensor("ident", [P, P], F32, kind="ExternalInput").ap()
    w1t_d = nc.dram_tensor("w1t", [P, FC * H], F32R, kind="ExternalInput").ap()
    w2t_d = nc.dram_tensor("w2t", [P, HC * H], F32R, kind="ExternalInput").ap()
    wlt_d = nc.dram_tensor("wlt", [P, HC * NL], F32, kind="ExternalInput").ap()
    b1_d = nc.dram_tensor("b1", [P, HC], F32, kind="ExternalInput").ap()
    b2_d = nc.dram_tensor("b2", [P, HC], F32, kind="ExternalInput").ap()
    bl_d = nc.dram_tensor("bl", [NL, 1], F32, kind="ExternalInput").ap()
    msk_d = nc.dram_tensor("msk", [P, B_LOC * S], F32, kind="ExternalInput").ap()
    out_d = nc.dram_tensor("out", [B_LOC, NL], F32, kind="ExternalOutput").ap()

    with TileKernel(nc) as (tc, ctx):
        const = ctx.enter_context(tc.tile_pool(name="const", bufs=1))
        xpool = ctx.enter_context(tc.tile_pool(name="x", bufs=18))
        hpool = ctx.enter_context(tc.tile_pool(name="h", bufs=3))
        htpool = ctx.enter_context(tc.tile_pool(name="ht", bufs=2))
        apool = ctx.enter_context(tc.tile_pool(name="acts", bufs=2))
        spool = ctx.enter_context(tc.tile_pool(name="small", bufs=4))
        pmix0 = ctx.enter_context(tc.tile_pool(name="pmix0", bufs=2, space="PSUM"))
        pmix1 = ctx.enter_context(tc.tile_pool(name="pmix1", bufs=2, space="PSUM"))
        ptr = ctx.enter_context(tc.tile_pool(name="ptr", bufs=2, space="PSUM"))
        pout = ctx.enter_context(tc.tile_pool(name="pout", bufs=2, space="PSUM"))

        # ---- constants into SBUF (one DMA each) ----
        seye = const.tile([P, L * P], F32R)
        nc.sync.dma_start(seye[:], seye_d[:])
        ident = const.tile([P, P], F32)
        nc.sync.dma_start(ident[:], ident_d[:])
        w1t = const.tile([P, FC * H], F32R)
        nc.sync.dma_start(w1t[:], w1t_d[:])
        w2t = const.tile([P, HC * H], F32R)
        nc.sync.dma_start(w2t[:], w2t_d[:])
        wlt = const.tile([P, HC * NL], F32)
        nc.sync.dma_start(wlt[:], wlt_d[:])
        b1 = const.tile([P, HC], F32)
        nc.sync.dma_start(b1[:], b1_d[:])
        b2 = const.tile([P, HC], F32)
        nc.sync.dma_start(b2[:], b2_d[:])
        bl = const.tile([NL, 1], F32)
        nc.sync.dma_start(bl[:], bl_d[:])
        msk = const.tile([P, B_LOC * S], F32)
        nc.sync.dma_start(msk[:], msk_d[:])

        logits = const.tile([NL, B_LOC], F32)

        for b in range(B_LOC):
            # hT[fc block of 1024 cols] = transposed mixed features
            hT = htpool.tile([P, FC * S], F32R, tag="hT")

            for ti in range(TT):
                pm0 = pmix0.tile([P, 512], F32, tag="pm0")
                pm1 = pmix1.tile([P, F - 512], F32, tag="pm1")
                for l in range(n_layers):
                    xt = xpool.tile([P, F], F32R, tag="xt")
                    nc.sync.dma_start(xt[:], x_d[b, l, ti * P:(ti + 1) * P, :])
                    se = seye[:, l * P:(l + 1) * P]
                    st, sp = (l == 0), (l == n_layers - 1)
                    nc.tensor.matmul(pm0[:], _r(se), _r(xt[:, 0:512]),
                                     start=st, stop=sp)
                    nc.tensor.matmul(pm1[:], _r(se), _r(xt[:, 512:F]),
                                     start=st, stop=sp)
                # PSUM -> SBUF mixed tile
                h = hpool.tile([P, F], F32, tag="h")
                nc.scalar.copy(h[:, 0:512], pm0[:])
                nc.scalar.copy(h[:, 512:F], pm1[:])
                # transpose 128x128 blocks into hT
                for fc in range(FC):
                    pt = ptr.tile([P, P], F32, tag="pt")
                    nc.tensor.transpose(pt[:], h[:, fc * P:(fc + 1) * P], ident[:])
                    dst = hT[:, fc * S + ti * P: fc * S + (ti + 1) * P]
                    if fc % 2 == 0:
                        nc.scalar.copy(dst, pt[:])
                    else:
                        nc.vector.tensor_copy(dst, pt[:])

            # ---- MLP layer 1: h1[m][t] = relu(W1T[:,m].T @ hT + b1) ----
            h1 = apool.tile([P, HC * S], F32R, tag="h1")
            for m in range(HC):
                for n in range(S // 512):
                    o1 = pout.tile([P, 512], F32, tag="po")
                    for k in range(FC):
                        lhs = w1t[:, k * H + m * P: k * H + (m + 1) * P]
                        rhs = hT[:, k * S + n * 512: k * S + (n + 1) * 512]
                        nc.tensor.matmul(o1[:], _r(lhs), _r(rhs),
                                         start=(k == 0), stop=(k == FC - 1))
                    nc.scalar.activation(
                        h1[:, m * S + n * 512: m * S + (n + 1) * 512], o1[:],
                        mybir.ActivationFunctionType.Relu,
                        bias=b1[:, m:m + 1], scale=1.0)

            # ---- MLP layer 2 ----
            h2 = apool.tile([P, HC * S], F32, tag="h2")
            for m in range(HC):
                for n in range(S // 512):
                    o2 = pout.tile([P, 512], F32, tag="po")
                    for k in range(HC):
                        lhs = w2t[:, k * H + m * P: k * H + (m + 1) * P]
                        rhs = h1[:, k * S + n * 512: k * S + (n + 1) * 512]
                        nc.tensor.matmul(o2[:], _r(lhs), _r(rhs),
                                         start=(k == 0), stop=(k == HC - 1))
                    nc.scalar.activation(
                        h2[:, m * S + n * 512: m * S + (n + 1) * 512], o2[:],
                        mybir.ActivationFunctionType.Relu,
                        bias=b2[:, m:m + 1], scale=1.0)

            # ---- masked mean pool + logits ----
            plog = pout.tile([NL, 1], F32, tag="po")
            for m in range(HC):
                junk = spool.tile([P, S], F32, tag="junk")
                pooled = spool.tile([P, 1], F32, tag="pooled")
                nc.vector.scalar_tensor_tensor(
                    out=junk[:], in0=h2[:, m * S:(m + 1) * S],
                    scalar=1.0, in1=msk[:, b * S:(b + 1) * S],
                    op0=mybir.AluOpType.bypass, op1=mybir.AluOpType.mult,
                    accum_out=pooled[:])
                nc.tensor.matmul(plog[:], _r(wlt[:, m * NL:(m + 1) * NL]),
                                 _r(pooled[:]),
                                 start=(m == 0), stop=(m == HC - 1))
            nc.vector.tensor_tensor(logits[:, b:b + 1], plog[:], bl[:],
                                    mybir.AluOpType.add)

        nc.sync.dma_start(out_d.rearrange("o f -> f o"), logits[:])

    if split_waits:
        _split_excess_waits(nc, max_waits=1)
    return nc


class TileKernel:
    """TileContext + ExitStack in one `with`."""

    def __init__(self, nc):
        self.tc = tile.TileContext(nc)
        self.ctx = ExitStack()

    def __enter__(self):
        tc = self.tc.__enter__()
        self.ctx.__enter__()
        return tc, self.ctx

    def __exit__(self, *exc):
        self.ctx.__exit__(*exc)
        return self.tc.__exit__(*exc)


_PROGRAM_CACHE: dict[int, bass.Bass] = {}


def _get_program(n_layers: int) -> bass.Bass:
    if n_layers not in _PROGRAM_CACHE:
        _PROGRAM_CACHE[n_layers] = build_program(n_layers)
    return _PROGRAM_CACHE[n_layers]


def _softmax32(v: np.ndarray) -> np.ndarray:
    v = v.astype(np.float32)
    e = np.exp(v - v.max())
    return (e / e.sum()).astype(np.float32)


def _prep_in_maps(inputs: dict) -> list[dict]:
    x = np.asarray(inputs["x"])
    lengths = np.asarray(inputs["lengths"])

    # host-side prep of the small replicated operands
    s = (_softmax32(np.asarray(inputs["mixing_weights"]))
         * np.float32(np.asarray(inputs["gamma"]).reshape(-1)[0]))
    seye = np.zeros((P, L * P), np.float32)
    for l in range(L):
        seye[:, l * P:(l + 1) * P] = np.eye(P, dtype=np.float32) * s[l]
    ident = np.eye(P, dtype=np.float32)

    W1 = np.asarray(inputs["W1"], np.float32)  # [H, F]
    W2 = np.asarray(inputs["W2"], np.float32)  # [H, H]
    Wl = np.asarray(inputs["Wl"], np.float32)  # [NL, H]
    w1t = np.ascontiguousarray(
        W1.T.reshape(FC, P, H).transpose(1, 0, 2).reshape(P, FC * H))
    w2t = np.ascontiguousarray(
        W2.T.reshape(HC, P, H).transpose(1, 0, 2).reshape(P, HC * H))
    wlt = np.ascontiguousarray(
        Wl.T.reshape(HC, P, NL).transpose(1, 0, 2).reshape(P, HC * NL))
    b1p = np.ascontiguousarray(np.asarray(inputs["b1"], np.float32).reshape(HC, P).T)
    b2p = np.ascontiguousarray(np.asarray(inputs["b2"], np.float32).reshape(HC, P).T)
    blp = np.asarray(inputs["bl"], np.float32).reshape(NL, 1)

    in_maps = []
    for c in range(N_CORES):
        sl = slice(c * B_LOC, (c + 1) * B_LOC)
        lens = lengths[sl].astype(np.float32)
        msk = np.zeros((P, B_LOC * S), np.float32)
        for b in range(B_LOC):
            msk[:, b * S:(b + 1) * S] = (
                (np.arange(S, dtype=np.float32) < lens[b]) / lens[b])[None, :]
        in_maps.append({
            "x": np.ascontiguousarray(x[sl]),
            "seye": seye, "ident": ident,
            "w1t": w1t, "w2t": w2t, "wlt": wlt,
            "b1": b1p, "b2": b2p, "bl": blp,
            "msk": msk,
        })
    return in_maps


def kernel(x, lengths, layer, gamma, mixing_weights, W1, b1, W2, b2, Wl, bl):
    n_layers = int(np.asarray(layer)) + 1
    assert 1 <= n_layers <= L

    nc = _get_program(n_layers)
    in_maps = _prep_in_maps(dict(
        x=x, lengths=lengths, gamma=gamma, mixing_weights=mixing_weights,
        W1=W1, b1=b1, W2=W2, b2=b2, Wl=Wl, bl=bl))

    res = run_bass_kernel_spmd(nc, in_maps, list(range(N_CORES)))
    return np.concatenate([res.results[c]["out"] for c in range(N_CORES)], axis=0)


# revision 23
# speedup vs baseline: 646.6358x; 646.6358x over previous
"""Trainium2 Bass kernel for nn_CumulativeProbingDense.

Computation (see reference):
    h      = sum_l softmax(mixing_weights)[l] * x[:, l] * gamma   # [B, S, F]
    h1     = relu(h @ W1.T + b1)                                  # [B, S, H]
    h2     = relu(h1 @ W2.T + b2)                                 # [B, S, H]
    pooled = (h2 * mask).sum(S) / lengths                         # [B, H]
    logits = pooled @ Wl.T + bl                                   # [B, NL]

Sharding: pure data parallel over batch, 2 samples per core on 8 cores.
The dominant cost is streaming x (654 MB fp32) from HBM once.

Device strategy per core:
  - layer mix on the TensorE: PSUM-accumulated matmuls with a scaled
    identity as the stationary operand and x tiles (natural [token, feat]
    layout) as the moving operand -> h tile [128 t, 768 f]
  - PE transpose h tiles into hT [feat partitions, token free]
  - MLP matmuls with W1T/W2T chunks stationary, relu+bias on ScalarE
  - masked mean-pool with one fused DVE tensor_tensor_reduce against a
    host-prebuilt mask/length tile, then a tiny matmul for the logits
"""

import numpy as np

import concourse.bass as bass
import concourse.tile as tile
from concourse import mybir
from concourse.bass_utils import run_bass_kernel_spmd
from contextlib import ExitStack

F32 = mybir.dt.float32
F32R = mybir.dt.float32r

N_CORES = 8
B, L, S, F = 16, 13, 1024, 768
H, NL = 256, 7
B_LOC = B // N_CORES          # samples per core
P = 128                       # SBUF partitions
TT = S // P                   # token tiles per sample
FC = F // P                   # feature chunks of 128
HC = H // P                   # hidden chunks of 128

# matmul input dtype: float32r streams at 1 cycle/row (moving dim >= 256)
# vs plain float32's 4 cycles/row. fp32r rounds the operands (TF32-like),
# measured end-to-end rel err vs the fp32 reference on hardware: ~1.8e-4.
MM_DT = F32R


def _split_excess_waits(nc, max_waits=1):
    """walrus (CoreV3) rejects instructions carrying more than a couple of
    sync waits (e.g. the TileContext exit drain). Hoist excess waits onto
    standalone NoOps inserted before the offending instruction."""
    n_fixed = 0
    for f in nc.m.functions:
        for bb in f.blocks:
            out, changed = [], False
            for inst in bb.instructions:
                si = getattr(inst, "sync_info", None)
                if si is not None and len(si.on_wait) > max_waits:
                    waits = list(si.on_wait)
                    for j, w in enumerate(waits[max_waits:]):
                        out.append(mybir.InstNoOp(
                            name=f"{inst.name}-wsplit{j}",
                            engine=inst.engine, ins=[], outs=[],
                            sync_info=mybir.SyncInfo(on_wait=[w], on_update=[]),
                        ))
                    inst.sync_info = mybir.SyncInfo(
                        on_wait=waits[:max_waits], on_update=list(si.on_update))
                    changed = True
                    n_fixed += 1
                out.append(inst)
            if changed:
                bb.instructions = out
    return n_fixed


def _r(ap):
    return ap


def build_program(n_layers: int, split_waits: bool = True, repeat: int = 1,
                  batched_dma: bool = True,
                  hw_loop_repeat: int | None = None,
                  mix_dve_layers: int = 0) -> bass.Bass:
    # mix_dve_layers: how many of the trailing layers are accumulated on the
    # DVE (axpy) instead of the TensorE, to balance PE vs DVE occupancy.
    n_pe_layers = n_layers - mix_dve_layers
    assert n_pe_layers >= 1
    nc = bass.Bass("TRN2", target_bir_lowering=False, debug=False, num_devices=1)

    x_d = nc.dram_tensor("x", [B_LOC, L, S, F], F32R, kind="ExternalInput").ap()
    seye_d = nc.dram_tensor("seye", [P, L * P], F32R, kind="ExternalInput").ap()
    ident_d = nc.dram_tensor("ident", [P, P], F32, kind="ExternalInput").ap()
    w1t_d = nc.dram_tensor("w1t", [P, FC * H], F32R, kind="ExternalInput").ap()
    w2t_d = nc.dram_tensor("w2t", [P, HC * H], F32R, kind="ExternalInput").ap()
    wlt_d = nc.dram_tensor("wlt", [P, HC * NL], F32, kind="ExternalInput").ap()
    b1_d = nc.dram_tensor("b1", [P, HC], F32, kind="ExternalInput").ap()
    b2_d = nc.dram_tensor("b2", [P, HC], F32, kind="ExternalInput").ap()
    bl_d = nc.dram_tensor("bl", [NL, 1], F32, kind="ExternalInput").ap()
    msk_d = nc.dram_tensor("msk", [P, B_LOC * S], F32, kind="ExternalInput").ap()
    svec_d = nc.dram_tensor("svec", [P, L], F32, kind="ExternalInput").ap()
    out_d = nc.dram_tensor("out", [B_LOC, NL], F32, kind="ExternalOutput").ap()

    with TileKernel(nc) as (tc, ctx):
        const = ctx.enter_context(tc.tile_pool(name="const", bufs=1))
        xpool = ctx.enter_context(tc.tile_pool(name="x", bufs=2 if batched_dma else 18))
        hpool = ctx.enter_context(tc.tile_pool(name="h", bufs=3))
        htpool = ctx.enter_context(tc.tile_pool(name="ht", bufs=2))
        apool = ctx.enter_context(tc.tile_pool(name="acts", bufs=1 if batched_dma else 2))
        spool = ctx.enter_context(tc.tile_pool(name="small", bufs=2 if batched_dma else 4))
        pmix0 = ctx.enter_context(tc.tile_pool(name="pmix0", bufs=2, space="PSUM"))
        pmix1 = ctx.enter_context(tc.tile_pool(name="pmix1", bufs=2, space="PSUM"))
        ptr = ctx.enter_context(tc.tile_pool(name="ptr", bufs=2, space="PSUM"))
        pout = ctx.enter_context(tc.tile_pool(name="pout", bufs=2, space="PSUM"))

        # ---- constants into SBUF via SWDGE (gpsimd), keeping both HWDGE
        # rings free for the x stream ----
        seye = const.tile([P, L * P], F32R)
        nc.gpsimd.dma_start(seye[:], seye_d[:])
        ident = const.tile([P, P], F32)
        nc.gpsimd.dma_start(ident[:], ident_d[:])
        w1t = const.tile([P, FC * H], F32R)
        nc.gpsimd.dma_start(w1t[:], w1t_d[:])
        w2t = const.tile([P, HC * H], F32R)
        nc.gpsimd.dma_start(w2t[:], w2t_d[:])
        wlt = const.tile([P, HC * NL], F32)
        nc.gpsimd.dma_start(wlt[:], wlt_d[:])
        b1 = const.tile([P, HC], F32)
        nc.gpsimd.dma_start(b1[:], b1_d[:])
        b2 = const.tile([P, HC], F32)
        nc.gpsimd.dma_start(b2[:], b2_d[:])
        bl = const.tile([NL, 1], F32)
        nc.gpsimd.dma_start(bl[:], bl_d[:])
        msk = const.tile([P, B_LOC * S], F32)
        nc.gpsimd.dma_start(msk[:], msk_d[:])
        svec = const.tile([P, L], F32)
        nc.gpsimd.dma_start(svec[:], svec_d[:])

        logits = const.tile([NL, B_LOC], F32)

        CW = 256                # token width of one streamed MLP chunk
        NCH = S // CW           # chunks per sample

        def mlp_chunk(b, n, hT, h1, h2):
            """mm1 + mm2 + relus for token chunk n (cols n*CW..(n+1)*CW)."""
            for m in range(HC):
                o1 = pout.tile([P, CW], F32, tag="po")
                for k in range(FC):
                    lhs = w1t[:, k * H + m * P: k * H + (m + 1) * P]
                    rhs = hT[:, k * S + n * CW: k * S + (n + 1) * CW]
                    nc.tensor.matmul(o1[:], lhs, rhs,
                                     start=(k == 0), stop=(k == FC - 1))
                nc.scalar.activation(
                    h1[:, m * S + n * CW: m * S + (n + 1) * CW], o1[:],
                    mybir.ActivationFunctionType.Relu,
                    bias=b1[:, m:m + 1], scale=1.0)
            for m in range(HC):
                o2 = pout.tile([P, CW], F32, tag="po")
                for k in range(HC):
                    lhs = w2t[:, k * H + m * P: k * H + (m + 1) * P]
                    rhs = h1[:, k * S + n * CW: k * S + (n + 1) * CW]
                    nc.tensor.matmul(o2[:], lhs, rhs,
                                     start=(k == 0), stop=(k == HC - 1))
                nc.scalar.activation(
                    h2[:, m * S + n * CW: m * S + (n + 1) * CW], o2[:],
                    mybir.ActivationFunctionType.Relu,
                    bias=b2[:, m:m + 1], scale=1.0)

        def _body(_iv=None):
          for b in range(B_LOC):
            # hT[fc block of 1024 cols] = transposed mixed features
            hT = htpool.tile([P, FC * S], F32R, tag="hT")
            h1 = apool.tile([P, HC * S], F32R, tag="h1")
            h2 = apool.tile([P, HC * S], F32, tag="h2")

            for ti in range(TT):
                pm0 = pmix0.tile([P, 512], F32, tag="pm0")
                pm1 = pmix1.tile([P, F - 512], F32, tag="pm1")
                # All x DMAs ride the SP HWDGE ring (SP has no other work,
                # so triggers never queue behind compute). Two pieces per
                # token tile so the mix can start on the first piece while
                # the second is still in flight.
                n_a = (n_layers + 1) // 2
                xt13 = xpool.tile([P, n_layers, F], F32R, tag="xt")
                src_a = x_d[b, 0:n_a, ti * P:(ti + 1) * P, :] \
                    .rearrange("l t f -> t l f")
                nc.sync.dma_start(xt13[:, 0:n_a], src_a)
                if n_a < n_layers:
                    src_b = x_d[b, n_a:n_layers, ti * P:(ti + 1) * P, :] \
                        .rearrange("l t f -> t l f")
                    nc.sync.dma_start(xt13[:, n_a:n_layers], src_b)
                accd = None
                for l in range(n_layers):
                    xrow = xt13[:, l]
                    if l < n_pe_layers:
                        se = seye[:, l * P:(l + 1) * P]
                        st, sp = (l == 0), (l == n_pe_layers - 1)
                        nc.tensor.matmul(pm0[:], se, xrow[:, 0:512],
                                         start=st, stop=sp)
                        nc.tensor.matmul(pm1[:], se, xrow[:, 512:F],
                                         start=st, stop=sp)
                    else:
                        xf = xrow.bitcast(F32)
                        sc = svec[:, l:l + 1]
                        if accd is None:
                            accd = hpool.tile([P, F], F32, tag="accd")
                            nc.vector.tensor_scalar_mul(accd[:], xf, sc)
                        else:
                            nc.vector.scalar_tensor_tensor(
                                accd[:], xf, sc, accd[:],
                                op0=mybir.AluOpType.mult, op1=mybir.AluOpType.add)
                # PSUM (+ DVE partial) -> SBUF mixed tile
                h = hpool.tile([P, F], F32, tag="h")
                if accd is None:
                    nc.scalar.copy(h[:, 0:512], pm0[:])
                    nc.scalar.copy(h[:, 512:F], pm1[:])
                else:
                    nc.vector.scalar_tensor_tensor(
                        h[:, 0:512], pm0[:], 1.0, accd[:, 0:512],
                        op0=mybir.AluOpType.bypass, op1=mybir.AluOpType.add)
                    nc.vector.scalar_tensor_tensor(
                        h[:, 512:F], pm1[:], 1.0, accd[:, 512:F],
                        op0=mybir.AluOpType.bypass, op1=mybir.AluOpType.add)
                # transpose 128x128 blocks into hT
                for fc in range(FC):
                    pt = ptr.tile([P, P], F32, tag="pt")
                    nc.tensor.transpose(pt[:], h[:, fc * P:(fc + 1) * P], ident[:])
                    dst = hT[:, fc * S + ti * P: fc * S + (ti + 1) * P]
                    if fc % 2 == 0 or accd is not None:
                        nc.scalar.copy(dst, pt[:])
                    else:
                        nc.vector.tensor_copy(dst, pt[:])
                # stream the MLP over finished 512-token chunks so only the
                # last chunk's matmuls remain after the final DMA
                if (ti + 1) % (TT // NCH) == 0:
                    mlp_chunk(b, (ti + 1) // (TT // NCH) - 1, hT, h1, h2)

            # ---- masked mean pool + logits ----
            plog = pout.tile([NL, 1], F32, tag="po")
            for m in range(HC):
                junk = spool.tile([P, S], F32, tag="junk")
                pooled = spool.tile([P, 1], F32, tag="pooled")
                nc.vector.scalar_tensor_tensor(
                    out=junk[:], in0=h2[:, m * S:(m + 1) * S],
                    scalar=1.0, in1=msk[:, b * S:(b + 1) * S],
                    op0=mybir.AluOpType.bypass, op1=mybir.AluOpType.mult,
                    accum_out=pooled[:])
                nc.tensor.matmul(plog[:], wlt[:, m * NL:(m + 1) * NL],
                                 pooled[:],
                                 start=(m == 0), stop=(m == HC - 1))
            nc.vector.tensor_tensor(logits[:, b:b + 1], plog[:], bl[:],
                                    mybir.AluOpType.add)

        if hw_loop_repeat is not None and hw_loop_repeat > 1:
            with tc.For_i(0, hw_loop_repeat, 1) as _i:
                _body(_i)
        else:
            for _rep in range(repeat):
                _body()

        nc.sync.dma_start(out_d.rearrange("o f -> f o"), logits[:])

    if split_waits:
        _split_excess_waits(nc, max_waits=1)
    return nc


class TileKernel:
    """TileContext + ExitStack in one `with`."""

    def __init__(self, nc):
        self.tc = tile.TileContext(nc)
        self.ctx = ExitStack()

    def __enter__(self):
        tc = self.tc.__enter__()
        self.ctx.__enter__()
        return tc, self.ctx

    def __exit__(self, *exc):
        self.ctx.__exit__(*exc)
        return self.tc.__exit__(*exc)


_PROGRAM_CACHE: dict[int, bass.Bass] = {}


def _get_program(n_layers: int) -> bass.Bass:
    if n_layers not in _PROGRAM_CACHE:
        _PROGRAM_CACHE[n_layers] = build_program(n_layers)
    return _PROGRAM_CACHE[n_layers]


def _softmax32(v: np.ndarray) -> np.ndarray:
    v = v.astype(np.float32)
    e = np.exp(v - v.max())
    return (e / e.sum()).astype(np.float32)


def _prep_in_maps(inputs: dict) -> list[dict]:
    x = np.asarray(inputs["x"])
    lengths = np.asarray(inputs["lengths"])

    # host-side prep of the small replicated operands
    s = (_softmax32(np.asarray(inputs["mixing_weights"]))
         * np.float32(np.asarray(inputs["gamma"]).reshape(-1)[0]))
    seye = np.zeros((P, L * P), np.float32)
    for l in range(L):
        seye[:, l * P:(l + 1) * P] = np.eye(P, dtype=np.float32) * s[l]
    ident = np.eye(P, dtype=np.float32)

    W1 = np.asarray(inputs["W1"], np.float32)  # [H, F]
    W2 = np.asarray(inputs["W2"], np.float32)  # [H, H]
    Wl = np.asarray(inputs["Wl"], np.float32)  # [NL, H]
    w1t = np.ascontiguousarray(
        W1.T.reshape(FC, P, H).transpose(1, 0, 2).reshape(P, FC * H))
    w2t = np.ascontiguousarray(
        W2.T.reshape(HC, P, H).transpose(1, 0, 2).reshape(P, HC * H))
    wlt = np.ascontiguousarray(
        Wl.T.reshape(HC, P, NL).transpose(1, 0, 2).reshape(P, HC * NL))
    b1p = np.ascontiguousarray(np.asarray(inputs["b1"], np.float32).reshape(HC, P).T)
    b2p = np.ascontiguousarray(np.asarray(inputs["b2"], np.float32).reshape(HC, P).T)
    blp = np.asarray(inputs["bl"], np.float32).reshape(NL, 1)

    in_maps = []
    for c in range(N_CORES):
        sl = slice(c * B_LOC, (c + 1) * B_LOC)
        lens = lengths[sl].astype(np.float32)
        msk = np.zeros((P, B_LOC * S), np.float32)
        for b in range(B_LOC):
            msk[:, b * S:(b + 1) * S] = (
                (np.arange(S, dtype=np.float32) < lens[b]) / lens[b])[None, :]
        in_maps.append({
            "x": np.ascontiguousarray(x[sl]),
            "seye": seye, "ident": ident,
            "w1t": w1t, "w2t": w2t, "wlt": wlt,
            "b1": b1p, "b2": b2p, "bl": blp,
            "msk": msk,
            "svec": np.ascontiguousarray(np.tile(s, (P, 1))),
        })
    return in_maps


def kernel(x, lengths, layer, gamma, mixing_weights, W1, b1, W2, b2, Wl, bl):
    n_layers = int(np.asarray(layer)) + 1
    assert 1 <= n_layers <= L

    nc = _get_program(n_layers)
    in_maps = _prep_in_maps(dict(
        x=x, lengths=lengths, gamma=gamma, mixing_weights=mixing_weights,
        W1=W1, b1=b1, W2=W2, b2=b2, Wl=Wl, bl=bl))

    res = run_bass_kernel_spmd(nc, in_maps, list(range(N_CORES)))
    return np.concatenate([res.results[c]["out"] for c in range(N_CORES)], axis=0)
